# revision 6
# baseline (speedup 1.0000x reference)
"""Inverse DWT (BackwardTransformLayer) Trainium2 Bass kernel — v2.

Math (polyphase form of the zero-interleaved circular FFT convolution):
  out[r, 2p+pi] = sum_{s=0..3} cD[pi,s]*D[r,(p-s)%M] + cA[pi,s]*A[r,(p-s)%M]
  cD[0,s] = w[7-2s]   cD[1,s] = w[6-2s]   cA[0,s] = w[2s]   cA[1,s] = -w[2s+1]

Sharding: data-parallel over rows; 512 rows per core on 8 NeuronCores.

Measured 96694 ns (REPS-slope method; all-fp32 predecessor: 235940 ns by
the same method), rel err 1.3e-3 vs the 2e-2 gate — essentially at the
~93us/core HBM floor (33.5 MB at ~360 GB/s). The wins, in order:
  - Per-PANEL output tiles [128 x 2048], DMA'd to HBM the moment each
    panel's two parity chains/evictions finish (instead of half-tile
    [128 x 4096] outputs gated on two panels). This single overlap fix was
    worth 30us (126965 -> 96694): finer store granularity keeps the DMA
    queue fed and shrinks the pipeline drain.
  - PE panels (10 of 16): f32r (tf32-like) matmuls of identity-scaled
    weights reading the fp32 input tiles directly via bitcast — 1 cyc/row
    instead of fp32's 4, no conversion pass. Tiles are F32R-typed and DMA'd
    via a bitcast source AP because the BIR verifier requires f32r matmul
    operands to be f32r-typed ("rounded"); every other reader bitcasts
    back to F32.
  - Engine rebalance: 6 DVE panels / 10 PE panels (the old 21/11 split left
    DVE as a ~190us critical path). DVE chains are fp16
    scalar_tensor_tensor MACs; note STT has no fast DVE uop — it runs 1x
    regardless of dtype (confirmed in CoreSim cost model AND by HW timing).
    Coefficients are compile-time immediates: the NEFF is specialized on
    the runtime wavelet values by kernel() (works for any wavelet).
  - ScalarE produces two fp16 copies per DVE panel (bufE at ext[c0], bufO
    at ext[c0+1], windows at even offsets) and evicts PE PSUM stride-2.
Tried and measured SLOWER (or neutral), do not redo without new evidence:
  - DVE/PE rebalance: NDVE=5: 140908, NDVE=7: 134879 (6 is the optimum;
    TimelineSim predicted the opposite order — its scheduling deltas do
    NOT transfer to HW).
  - Grouped-PE (taps outermost per tile-parity to amortize LDWEIGHTS):
    128671 — LDW is hidden by the PE engine queue.
  - tensor_scalar(4x-claimed)+tensor_tensor tree chains, input-DMA halving,
    prefix-from-HBM, INP/OUT_BUFS 3/3: CoreSim said 112.8us, HW said
    142.0us — the cost model's fast-mode table for TensorScalarPtr
    overestimates real HW.
  - Parity-granular DVE/PE assignment: 126-129us in sim, never beat
    whole-panel granularity.
"""

import os
import sys

import numpy as np

for _p in ("/opt/trn_rl_repo", "/root/.axon_site/_ro/trn_rl_repo"):
    if os.path.isdir(_p) and _p not in sys.path:
        sys.path.append(_p)

import concourse.bass as bass  # noqa: E402
import concourse.tile as tile  # noqa: E402
from concourse import bacc, mybir  # noqa: E402
from concourse.bass_utils import run_bass_kernel_spmd  # noqa: E402

F32 = mybir.dt.float32
F32R = mybir.dt.float32r
F16 = mybir.dt.float16
COPY = mybir.ActivationFunctionType.Copy
MUL = mybir.AluOpType.mult
ADD = mybir.AluOpType.add

N_CORES = 8
P = 128          # partitions
M = 4096         # input row length
ROWS = 512       # rows per core
NT = ROWS // P   # row tiles per core
WU = 1024        # panel width (input cols)
NPAN = M // WU   # panels per tile
NCHUNK = 512     # psum chunk (one bank of fp32)

NDVE = int(os.environ.get("DWT_NDVE", "6"))     # panels (of NT*NPAN=16) on DVE
REPS = int(os.environ.get("DWT_REPS", "1"))     # benchmark-only in-kernel loop
IO_ONLY = bool(int(os.environ.get("DWT_IO_ONLY", "0")))
EVICT_ENG = os.environ.get("DWT_EVICT", "scalar")  # psum eviction engine
CVT_ENG = os.environ.get("DWT_CVT", "scalar")      # fp16 conversion engine


def _dve_panels():
    # Spread DVE panels across tiles and halves.
    order = [(t, p) for p in (1, 3, 0, 2) for t in range(NT)]
    return set(order[:NDVE])


def build_nc(dve_set, wavelet_vals=None):
    if wavelet_vals is None:
        # DB4 defaults (reference.setup_inputs uses these); kernel() always
        # rebuilds with the actual runtime wavelet on first call.
        wavelet_vals = np.array([-0.010597401784997278, 0.032883011666982945,
                                 0.030841381835986965, -0.18703481171888114,
                                 -0.02798376941698385, 0.6308807679295904,
                                 0.7148465705525415, 0.23037781330885523],
                                dtype=np.float64)
    wv64 = [float(v) for v in np.asarray(wavelet_vals, dtype=np.float64)]
    nc = bacc.Bacc()
    det = nc.declare_dram_parameter("details", [ROWS, M], F32, isOutput=False)
    app = nc.declare_dram_parameter("approximation", [ROWS, M], F32, isOutput=False)
    wav = nc.declare_dram_parameter("wavelet", [8], F32, isOutput=False)
    res = nc.declare_dram_parameter("result", [ROWS, 2 * M], F32, isOutput=True)
    ident = nc.inline_tensor(np.eye(P, dtype=np.float32), "ident")

    with tile.TileContext(nc) as tc:
        with (
            tc.tile_pool(name="const", bufs=1) as constp,
            tc.tile_pool(name="ine", bufs=2) as inp,
            tc.tile_pool(name="oute", bufs=6) as outp,
            tc.tile_pool(name="cvt", bufs=8) as cvtp,
            tc.tile_pool(name="acc", bufs=4) as accp,
            tc.tile_pool(name="psum", bufs=8, space="PSUM") as psump,
        ):
            # ---- coefficients as compile-time immediates: the scalar op
            # then lowers to TensorScalar (not TensorScalarPtr), whose 2x/4x
            # DVE uops exist; the Ptr variant runs at 1x. The NEFF is
            # specialized on the runtime wavelet values by kernel().
            # Token read keeps the "wavelet" ExternalInput alive in the NEFF.
            wv = constp.tile([1, 8], F32)
            nc.sync.dma_start(wv[:], wav[None, :])

            def coeff(x, pi, s):
                # x: 0 = details, 1 = approximation; pi: 0 = even, 1 = odd
                if x == 0:
                    return wv64[7 - 2 * s] if pi == 0 else wv64[6 - 2 * s]
                if pi == 0:
                    return wv64[2 * s]
                return -wv64[2 * s + 1]

            # ---- PE weights: c * I for each (input, parity, tap); F32R-typed
            # so the BIR verifier accepts them as f32r matmul operands.
            it = constp.tile([P, P], F32)
            nc.sync.dma_start(it[:], ident[:, :])
            w16 = constp.tile([P, 16 * P], F32R)

            def wslice(x, pi, s):
                j = (x * 2 + pi) * 4 + s
                return w16[:, j * P : (j + 1) * P]

            for x in range(2):
                for pi in range(2):
                    for s in range(4):
                        nc.vector.tensor_scalar(
                            wslice(x, pi, s), it[:], coeff(x, pi, s), None, MUL
                        )

            taps = [(x, s) for x in range(2) for s in range(4)]
            cvt_op = nc.scalar.copy if CVT_ENG == "scalar" else nc.vector.tensor_copy
            evict_op = nc.scalar.copy if EVICT_ENG == "scalar" else nc.vector.tensor_copy

            def body(_i=None):
              for t in range(NT):
                r0 = t * P
                # F32R-typed so PE can consume windows directly; every
                # non-PE reader bitcasts back to F32 (same bits).
                dext = inp.tile([P, M + 3], F32R, tag="dext")
                nc.sync.dma_start(dext[:, 3 : M + 3], det[r0 : r0 + P, :].bitcast(F32R))
                aext = inp.tile([P, M + 3], F32R, tag="aext")
                nc.sync.dma_start(aext[:, 3 : M + 3], app[r0 : r0 + P, :].bitcast(F32R))
                nc.vector.tensor_copy(dext[:, 0:3], dext[:, M : M + 3])
                nc.vector.tensor_copy(aext[:, 0:3], aext[:, M : M + 3])
                ext = [dext, aext]

                def xf(x, a, b):
                    return ext[x][:, a:b].bitcast(F32)

                for p in range(NPAN):  # one output tile per panel
                    if True:
                        c0 = p * WU
                        oh = outp.tile([P, 2 * WU], F32, tag="out",
                                       name=f"oh_{t}_{p}")
                        if IO_ONLY:
                            for pi in range(2):
                                nc.scalar.copy(
                                    oh[:, pi : 2 * WU : 2],
                                    xf(0, 3 + c0, 3 + c0 + WU),
                                )
                            continue
                        if (t, p) in dve_set:
                            # fp16 aligned copies: bufE = ext[c0:...], bufO = ext[c0+1:...]
                            bE, bO = [], []
                            for x in range(2):
                                be = cvtp.tile([P, WU + 2], F16, tag="cvt")
                                cvt_op(be[:], xf(x, c0, c0 + WU + 2))
                                bo = cvtp.tile([P, WU + 2], F16, tag="cvt")
                                cvt_op(bo[:], xf(x, c0 + 1, c0 + WU + 3))
                                bE.append(be)
                                bO.append(bo)

                            def win(x, s):
                                # tap window = ext[3-s+c0 : 3-s+c0+WU]; bufE holds
                                # ext[c0:...], bufO holds ext[c0+1:...] — offsets
                                # 3-s resp. 2-s are always even (4B-aligned fp16).
                                if s in (1, 3):
                                    return bE[x][:, 3 - s : 3 - s + WU]
                                return bO[x][:, 2 - s : 2 - s + WU]

                            for pi in range(2):
                                oview = oh[:, pi : 2 * WU : 2]
                                chain = [(0, 3), (0, 1), (0, 2), (0, 0),
                                         (1, 3), (1, 1), (1, 2)]
                                acc = accp.tile([P, WU], F16, tag="acc")
                                x0, s0 = chain[0]
                                nc.vector.tensor_scalar(
                                    acc[:], win(x0, s0), coeff(x0, pi, s0), None, MUL
                                )
                                for x, s in chain[1:]:
                                    nc.vector.scalar_tensor_tensor(
                                        acc[:], win(x, s), coeff(x, pi, s),
                                        acc[:], MUL, ADD,
                                    )
                                nc.vector.scalar_tensor_tensor(
                                    oview, win(1, 0), coeff(1, pi, 0),
                                    acc[:], MUL, ADD,
                                )
                        else:  # PE panel: f32r identity-scaled matmuls
                            for pi in range(2):
                                ccs = list(range(c0, c0 + WU, NCHUNK))
                                pss = [
                                    psump.tile([P, NCHUNK], F32, tag="ps",
                                               name=f"ps_{t}_{pi}_{p}_{ci}")
                                    for ci in range(len(ccs))
                                ]
                                for j, (x, s) in enumerate(taps):
                                    w = wslice(x, pi, s)
                                    for ci, cc in enumerate(ccs):
                                        rhs = ext[x][:, 3 - s + cc : 3 - s + cc + NCHUNK]
                                        nc.tensor.matmul(
                                            pss[ci][:], w, rhs,
                                            start=(j == 0), stop=(j == len(taps) - 1),
                                        )
                                for ci, cc in enumerate(ccs):
                                    evb = 2 * (cc - c0) + pi
                                    evict_op(
                                        oh[:, evb : evb + 2 * NCHUNK - 1 : 2], pss[ci][:]
                                    )
                    nc.sync.dma_start(
                        res[r0 : r0 + P, 2 * c0 : 2 * c0 + 2 * WU], oh[:])

            if REPS == 1:
                body()
            else:
                with tc.For_i(0, REPS, 1) as _rv:
                    body(_rv)
    nc.finalize()
    return nc


_CACHE = {}


def _get_nc(wavelet):
    key = wavelet.tobytes()
    if _CACHE.get("key") != key:
        _CACHE["nc"] = build_nc(_dve_panels(), wavelet)
        _CACHE["key"] = key
    return _CACHE["nc"]


def kernel(details, approximation, wavelet):
    details = np.ascontiguousarray(np.asarray(details, dtype=np.float32))
    approximation = np.ascontiguousarray(np.asarray(approximation, dtype=np.float32))
    wavelet = np.ascontiguousarray(np.asarray(wavelet, dtype=np.float32))
    assert details.shape == (N_CORES * ROWS, M) and approximation.shape == details.shape
    assert wavelet.shape == (8,)

    in_maps = [
        {
            "details": details[c * ROWS : (c + 1) * ROWS],
            "approximation": approximation[c * ROWS : (c + 1) * ROWS],
            "wavelet": wavelet,
        }
        for c in range(N_CORES)
    ]
    trace = bool(int(os.environ.get("DWT_TRACE", "0")))
    r = run_bass_kernel_spmd(_get_nc(wavelet), in_maps, list(range(N_CORES)), trace=trace)
    _CACHE["last_results"] = r
    return np.concatenate([r.results[c]["result"] for c in range(N_CORES)], axis=0)


# revision 7
# speedup vs baseline: 1.0003x; 1.0003x over previous
"""Inverse DWT (BackwardTransformLayer) Trainium2 Bass kernel — v2.

Math (polyphase form of the zero-interleaved circular FFT convolution):
  out[r, 2p+pi] = sum_{s=0..3} cD[pi,s]*D[r,(p-s)%M] + cA[pi,s]*A[r,(p-s)%M]
  cD[0,s] = w[7-2s]   cD[1,s] = w[6-2s]   cA[0,s] = w[2s]   cA[1,s] = -w[2s+1]

Sharding: data-parallel over rows; 512 rows per core on 8 NeuronCores.

Measured 96694 ns (REPS-slope method; all-fp32 predecessor: 235940 ns by
the same method), rel err 1.3e-3 vs the 2e-2 gate — essentially at the
~93us/core HBM floor (33.5 MB at ~360 GB/s). The wins, in order:
  - Per-PANEL output tiles [128 x 2048], DMA'd to HBM the moment each
    panel's two parity chains/evictions finish (instead of half-tile
    [128 x 4096] outputs gated on two panels). This single overlap fix was
    worth 30us (126965 -> 96694): finer store granularity keeps the DMA
    queue fed and shrinks the pipeline drain.
  - PE panels (10 of 16): f32r (tf32-like) matmuls of identity-scaled
    weights reading the fp32 input tiles directly via bitcast — 1 cyc/row
    instead of fp32's 4, no conversion pass. Tiles are F32R-typed and DMA'd
    via a bitcast source AP because the BIR verifier requires f32r matmul
    operands to be f32r-typed ("rounded"); every other reader bitcasts
    back to F32.
  - Engine rebalance: 6 DVE panels / 10 PE panels (the old 21/11 split left
    DVE as a ~190us critical path). DVE chains are fp16
    scalar_tensor_tensor MACs: on real HW fp16 STT runs in 2x packed mode
    (proved by A/B: identical kernel with fp32 chains and no conversions
    measures 126904 vs 96694 — the CoreSim cost model wrongly says 1x).
    The 4B-alignment dance below is what keeps 2x legal on every window.
    Coefficients are compile-time immediates: the NEFF is specialized on
    the runtime wavelet values by kernel() (works for any wavelet).
  - ScalarE produces two fp16 copies per DVE panel (bufE at ext[c0], bufO
    at ext[c0+1], windows at even offsets) and evicts PE PSUM stride-2.
Tried and measured SLOWER (or neutral), do not redo without new evidence:
  - DVE/PE rebalance: NDVE=5: 140908, NDVE=7: 134879 (6 is the optimum;
    TimelineSim predicted the opposite order — its scheduling deltas do
    NOT transfer to HW).
  - Grouped-PE (taps outermost per tile-parity to amortize LDWEIGHTS):
    128671 — LDW is hidden by the PE engine queue.
  - tensor_scalar(4x-claimed)+tensor_tensor tree chains, input-DMA halving,
    prefix-from-HBM, INP/OUT_BUFS 3/3: CoreSim said 112.8us, HW said
    142.0us — the cost model's fast-mode table for TensorScalarPtr
    overestimates real HW.
  - Parity-granular DVE/PE assignment: 126-129us in sim, never beat
    whole-panel granularity.
"""

import os
import sys

import numpy as np

for _p in ("/opt/trn_rl_repo", "/root/.axon_site/_ro/trn_rl_repo"):
    if os.path.isdir(_p) and _p not in sys.path:
        sys.path.append(_p)

import concourse.bass as bass  # noqa: E402
import concourse.tile as tile  # noqa: E402
from concourse import bacc, mybir  # noqa: E402
from concourse.bass_utils import run_bass_kernel_spmd  # noqa: E402

F32 = mybir.dt.float32
F32R = mybir.dt.float32r
F16 = mybir.dt.float16
COPY = mybir.ActivationFunctionType.Copy
MUL = mybir.AluOpType.mult
ADD = mybir.AluOpType.add

N_CORES = 8
P = 128          # partitions
M = 4096         # input row length
ROWS = 512       # rows per core
NT = ROWS // P   # row tiles per core
WU = 1024        # panel width (input cols)
NPAN = M // WU   # panels per tile
NCHUNK = 512     # psum chunk (one bank of fp32)

NDVE = int(os.environ.get("DWT_NDVE", "6"))     # panels (of NT*NPAN=16) on DVE
REPS = int(os.environ.get("DWT_REPS", "1"))     # benchmark-only in-kernel loop
IO_ONLY = bool(int(os.environ.get("DWT_IO_ONLY", "0")))
EVICT_ENG = os.environ.get("DWT_EVICT", "scalar")  # psum eviction engine
CVT_ENG = os.environ.get("DWT_CVT", "scalar")      # fp16 conversion engine


def _dve_panels():
    # Spread DVE panels across tiles and halves.
    order = [(t, p) for p in (1, 3, 0, 2) for t in range(NT)]
    return set(order[:NDVE])


def build_nc(dve_set, wavelet_vals=None):
    if wavelet_vals is None:
        # DB4 defaults (reference.setup_inputs uses these); kernel() always
        # rebuilds with the actual runtime wavelet on first call.
        wavelet_vals = np.array([-0.010597401784997278, 0.032883011666982945,
                                 0.030841381835986965, -0.18703481171888114,
                                 -0.02798376941698385, 0.6308807679295904,
                                 0.7148465705525415, 0.23037781330885523],
                                dtype=np.float64)
    wv64 = [float(v) for v in np.asarray(wavelet_vals, dtype=np.float64)]
    nc = bacc.Bacc()
    det = nc.declare_dram_parameter("details", [ROWS, M], F32, isOutput=False)
    app = nc.declare_dram_parameter("approximation", [ROWS, M], F32, isOutput=False)
    wav = nc.declare_dram_parameter("wavelet", [8], F32, isOutput=False)
    res = nc.declare_dram_parameter("result", [ROWS, 2 * M], F32, isOutput=True)
    ident = nc.inline_tensor(np.eye(P, dtype=np.float32), "ident")

    with tile.TileContext(nc) as tc:
        with (
            tc.tile_pool(name="const", bufs=1) as constp,
            tc.tile_pool(name="ine", bufs=2) as inp,
            tc.tile_pool(name="oute", bufs=6) as outp,
            tc.tile_pool(name="cvt", bufs=8) as cvtp,
            tc.tile_pool(name="acc", bufs=4) as accp,
            tc.tile_pool(name="psum", bufs=8, space="PSUM") as psump,
        ):
            # ---- coefficients as compile-time immediates: the scalar op
            # then lowers to TensorScalar (not TensorScalarPtr), whose 2x/4x
            # DVE uops exist; the Ptr variant runs at 1x. The NEFF is
            # specialized on the runtime wavelet values by kernel().
            # Token read keeps the "wavelet" ExternalInput alive in the NEFF.
            wv = constp.tile([1, 8], F32)
            nc.sync.dma_start(wv[:], wav[None, :])

            def coeff(x, pi, s):
                # x: 0 = details, 1 = approximation; pi: 0 = even, 1 = odd
                if x == 0:
                    return wv64[7 - 2 * s] if pi == 0 else wv64[6 - 2 * s]
                if pi == 0:
                    return wv64[2 * s]
                return -wv64[2 * s + 1]

            # ---- PE weights: c * I for each (input, parity, tap); F32R-typed
            # so the BIR verifier accepts them as f32r matmul operands.
            it = constp.tile([P, P], F32)
            nc.sync.dma_start(it[:], ident[:, :])
            w16 = constp.tile([P, 16 * P], F32R)

            def wslice(x, pi, s):
                j = (x * 2 + pi) * 4 + s
                return w16[:, j * P : (j + 1) * P]

            for x in range(2):
                for pi in range(2):
                    for s in range(4):
                        nc.vector.tensor_scalar(
                            wslice(x, pi, s), it[:], coeff(x, pi, s), None, MUL
                        )

            taps = [(x, s) for x in range(2) for s in range(4)]
            cvt_op = nc.scalar.copy if CVT_ENG == "scalar" else nc.vector.tensor_copy
            evict_op = nc.scalar.copy if EVICT_ENG == "scalar" else nc.vector.tensor_copy

            def body(_i=None):
              for t in range(NT):
                r0 = t * P
                # F32R-typed so PE can consume windows directly; every
                # non-PE reader bitcasts back to F32 (same bits).
                dext = inp.tile([P, M + 3], F32R, tag="dext")
                nc.sync.dma_start(dext[:, 3 : M + 3], det[r0 : r0 + P, :].bitcast(F32R))
                aext = inp.tile([P, M + 3], F32R, tag="aext")
                nc.sync.dma_start(aext[:, 3 : M + 3], app[r0 : r0 + P, :].bitcast(F32R))
                nc.vector.tensor_copy(dext[:, 0:3], dext[:, M : M + 3])
                nc.vector.tensor_copy(aext[:, 0:3], aext[:, M : M + 3])
                ext = [dext, aext]

                def xf(x, a, b):
                    return ext[x][:, a:b].bitcast(F32)

                for p in range(NPAN):  # one output tile per panel
                    if True:
                        c0 = p * WU
                        oh = outp.tile([P, 2 * WU], F32, tag="out",
                                       name=f"oh_{t}_{p}")
                        if IO_ONLY:
                            for pi in range(2):
                                nc.scalar.copy(
                                    oh[:, pi : 2 * WU : 2],
                                    xf(0, 3 + c0, 3 + c0 + WU),
                                )
                            continue
                        if (t, p) in dve_set:
                            # fp16 aligned copies: bufE = ext[c0:...], bufO = ext[c0+1:...]
                            bE, bO = [], []
                            for x in range(2):
                                be = cvtp.tile([P, WU + 2], F16, tag="cvt")
                                cvt_op(be[:], xf(x, c0, c0 + WU + 2))
                                bo = cvtp.tile([P, WU + 2], F16, tag="cvt")
                                cvt_op(bo[:], xf(x, c0 + 1, c0 + WU + 3))
                                bE.append(be)
                                bO.append(bo)

                            def win(x, s):
                                # tap window = ext[3-s+c0 : 3-s+c0+WU]; bufE holds
                                # ext[c0:...], bufO holds ext[c0+1:...] — offsets
                                # 3-s resp. 2-s are always even (4B-aligned fp16).
                                if s in (1, 3):
                                    return bE[x][:, 3 - s : 3 - s + WU]
                                return bO[x][:, 2 - s : 2 - s + WU]

                            for pi in range(2):
                                oview = oh[:, pi : 2 * WU : 2]
                                chain = [(0, 3), (0, 1), (0, 2), (0, 0),
                                         (1, 3), (1, 1), (1, 2)]
                                acc = accp.tile([P, WU], F16, tag="acc")
                                x0, s0 = chain[0]
                                nc.vector.tensor_scalar(
                                    acc[:], win(x0, s0), coeff(x0, pi, s0), None, MUL
                                )
                                for x, s in chain[1:]:
                                    nc.vector.scalar_tensor_tensor(
                                        acc[:], win(x, s), coeff(x, pi, s),
                                        acc[:], MUL, ADD,
                                    )
                                nc.vector.scalar_tensor_tensor(
                                    oview, win(1, 0), coeff(1, pi, 0),
                                    acc[:], MUL, ADD,
                                )
                        else:  # PE panel: f32r identity-scaled matmuls
                            for pi in range(2):
                                ccs = list(range(c0, c0 + WU, NCHUNK))
                                pss = [
                                    psump.tile([P, NCHUNK], F32, tag="ps",
                                               name=f"ps_{t}_{pi}_{p}_{ci}")
                                    for ci in range(len(ccs))
                                ]
                                for j, (x, s) in enumerate(taps):
                                    w = wslice(x, pi, s)
                                    for ci, cc in enumerate(ccs):
                                        rhs = ext[x][:, 3 - s + cc : 3 - s + cc + NCHUNK]
                                        nc.tensor.matmul(
                                            pss[ci][:], w, rhs,
                                            start=(j == 0), stop=(j == len(taps) - 1),
                                        )
                                for ci, cc in enumerate(ccs):
                                    evb = 2 * (cc - c0) + pi
                                    evict_op(
                                        oh[:, evb : evb + 2 * NCHUNK - 1 : 2], pss[ci][:]
                                    )
                    nc.sync.dma_start(
                        res[r0 : r0 + P, 2 * c0 : 2 * c0 + 2 * WU], oh[:])

            if REPS == 1:
                body()
            else:
                with tc.For_i(0, REPS, 1) as _rv:
                    body(_rv)
    nc.finalize()
    return nc


_CACHE = {}


def _get_nc(wavelet):
    key = wavelet.tobytes()
    if _CACHE.get("key") != key:
        _CACHE["nc"] = build_nc(_dve_panels(), wavelet)
        _CACHE["key"] = key
    return _CACHE["nc"]


def kernel(details, approximation, wavelet):
    details = np.ascontiguousarray(np.asarray(details, dtype=np.float32))
    approximation = np.ascontiguousarray(np.asarray(approximation, dtype=np.float32))
    wavelet = np.ascontiguousarray(np.asarray(wavelet, dtype=np.float32))
    assert details.shape == (N_CORES * ROWS, M) and approximation.shape == details.shape
    assert wavelet.shape == (8,)

    in_maps = [
        {
            "details": details[c * ROWS : (c + 1) * ROWS],
            "approximation": approximation[c * ROWS : (c + 1) * ROWS],
            "wavelet": wavelet,
        }
        for c in range(N_CORES)
    ]
    trace = bool(int(os.environ.get("DWT_TRACE", "0")))
    r = run_bass_kernel_spmd(_get_nc(wavelet), in_maps, list(range(N_CORES)), trace=trace)
    _CACHE["last_results"] = r
    return np.concatenate([r.results[c]["result"] for c in range(N_CORES)], axis=0)


# revision 8
# speedup vs baseline: 1.0044x; 1.0042x over previous
"""Inverse DWT (BackwardTransformLayer) Trainium2 Bass kernel — v2.

Math (polyphase form of the zero-interleaved circular FFT convolution):
  out[r, 2p+pi] = sum_{s=0..3} cD[pi,s]*D[r,(p-s)%M] + cA[pi,s]*A[r,(p-s)%M]
  cD[0,s] = w[7-2s]   cD[1,s] = w[6-2s]   cA[0,s] = w[2s]   cA[1,s] = -w[2s+1]

Sharding: data-parallel over rows; 512 rows per core on 8 NeuronCores.

Measured 96694 ns (REPS-slope method; all-fp32 predecessor: 235940 ns by
the same method), rel err 1.3e-3 vs the 2e-2 gate — essentially at the
~93us/core HBM floor (33.5 MB at ~360 GB/s). The wins, in order:
  - Per-PANEL output tiles [128 x 2048], DMA'd to HBM the moment each
    panel's two parity chains/evictions finish (instead of half-tile
    [128 x 4096] outputs gated on two panels). This single overlap fix was
    worth 30us (126965 -> 96694): finer store granularity keeps the DMA
    queue fed and shrinks the pipeline drain. The optimum is sharp:
    half-tile 16KB stores = 126965, panel 8KB = 96694, 4KB slices =
    126872, and splitting INPUT loads the same way = 119725. Touch
    neither side's granularity.
  - PE panels (10 of 16): f32r (tf32-like) matmuls of identity-scaled
    weights reading the fp32 input tiles directly via bitcast — 1 cyc/row
    instead of fp32's 4, no conversion pass. Tiles are F32R-typed and DMA'd
    via a bitcast source AP because the BIR verifier requires f32r matmul
    operands to be f32r-typed ("rounded"); every other reader bitcasts
    back to F32.
  - Engine rebalance: 6 DVE panels / 10 PE panels (the old 21/11 split left
    DVE as a ~190us critical path). DVE chains are fp16
    scalar_tensor_tensor MACs: on real HW fp16 STT runs in 2x packed mode
    (proved by A/B: identical kernel with fp32 chains and no conversions
    measures 126904 vs 96694 — the CoreSim cost model wrongly says 1x).
    The 4B-alignment dance below is what keeps 2x legal on every window.
    Coefficients are compile-time immediates: the NEFF is specialized on
    the runtime wavelet values by kernel() (works for any wavelet).
  - ScalarE produces two fp16 copies per DVE panel (bufE at ext[c0], bufO
    at ext[c0+1], windows at even offsets) and evicts PE PSUM stride-2.
Tried and measured SLOWER (or neutral), do not redo without new evidence:
  - DVE/PE rebalance: NDVE=5: 140908, NDVE=7: 134879 (6 is the optimum;
    TimelineSim predicted the opposite order — its scheduling deltas do
    NOT transfer to HW).
  - Grouped-PE (taps outermost per tile-parity to amortize LDWEIGHTS):
    128671 — LDW is hidden by the PE engine queue.
  - tensor_scalar(4x-claimed)+tensor_tensor tree chains, input-DMA halving,
    prefix-from-HBM, INP/OUT_BUFS 3/3: CoreSim said 112.8us, HW said
    142.0us — the cost model's fast-mode table for TensorScalarPtr
    overestimates real HW.
  - Parity-granular DVE/PE assignment: 126-129us in sim, never beat
    whole-panel granularity.
"""

import os
import sys

import numpy as np

for _p in ("/opt/trn_rl_repo", "/root/.axon_site/_ro/trn_rl_repo"):
    if os.path.isdir(_p) and _p not in sys.path:
        sys.path.append(_p)

import concourse.bass as bass  # noqa: E402
import concourse.tile as tile  # noqa: E402
from concourse import bacc, mybir  # noqa: E402
from concourse.bass_utils import run_bass_kernel_spmd  # noqa: E402

F32 = mybir.dt.float32
F32R = mybir.dt.float32r
F16 = mybir.dt.float16
COPY = mybir.ActivationFunctionType.Copy
MUL = mybir.AluOpType.mult
ADD = mybir.AluOpType.add

N_CORES = 8
P = 128          # partitions
M = 4096         # input row length
ROWS = 512       # rows per core
NT = ROWS // P   # row tiles per core
WU = 1024        # panel width (input cols)
NPAN = M // WU   # panels per tile
NCHUNK = 512     # psum chunk (one bank of fp32)

NDVE = int(os.environ.get("DWT_NDVE", "6"))     # panels (of NT*NPAN=16) on DVE
REPS = int(os.environ.get("DWT_REPS", "1"))     # benchmark-only in-kernel loop
IO_ONLY = bool(int(os.environ.get("DWT_IO_ONLY", "0")))
EVICT_ENG = os.environ.get("DWT_EVICT", "scalar")  # psum eviction engine
CVT_ENG = os.environ.get("DWT_CVT", "scalar")      # fp16 conversion engine


def _dve_panels():
    # Spread DVE panels across tiles and halves.
    order = [(t, p) for p in (1, 3, 0, 2) for t in range(NT)]
    return set(order[:NDVE])


def build_nc(dve_set, wavelet_vals=None):
    if wavelet_vals is None:
        # DB4 defaults (reference.setup_inputs uses these); kernel() always
        # rebuilds with the actual runtime wavelet on first call.
        wavelet_vals = np.array([-0.010597401784997278, 0.032883011666982945,
                                 0.030841381835986965, -0.18703481171888114,
                                 -0.02798376941698385, 0.6308807679295904,
                                 0.7148465705525415, 0.23037781330885523],
                                dtype=np.float64)
    wv64 = [float(v) for v in np.asarray(wavelet_vals, dtype=np.float64)]
    nc = bacc.Bacc()
    det = nc.declare_dram_parameter("details", [ROWS, M], F32, isOutput=False)
    app = nc.declare_dram_parameter("approximation", [ROWS, M], F32, isOutput=False)
    wav = nc.declare_dram_parameter("wavelet", [8], F32, isOutput=False)
    res = nc.declare_dram_parameter("result", [ROWS, 2 * M], F32, isOutput=True)
    ident = nc.inline_tensor(np.eye(P, dtype=np.float32), "ident")

    with tile.TileContext(nc) as tc:
        with (
            tc.tile_pool(name="const", bufs=1) as constp,
            tc.tile_pool(name="ine", bufs=2) as inp,
            tc.tile_pool(name="oute", bufs=6) as outp,
            tc.tile_pool(name="cvt", bufs=8) as cvtp,
            tc.tile_pool(name="acc", bufs=4) as accp,
            tc.tile_pool(name="psum", bufs=8, space="PSUM") as psump,
        ):
            # ---- coefficients as compile-time immediates: the scalar op
            # then lowers to TensorScalar (not TensorScalarPtr), whose 2x/4x
            # DVE uops exist; the Ptr variant runs at 1x. The NEFF is
            # specialized on the runtime wavelet values by kernel().
            # Token read keeps the "wavelet" ExternalInput alive in the NEFF.
            wv = constp.tile([1, 8], F32)
            nc.sync.dma_start(wv[:], wav[None, :])

            def coeff(x, pi, s):
                # x: 0 = details, 1 = approximation; pi: 0 = even, 1 = odd
                if x == 0:
                    return wv64[7 - 2 * s] if pi == 0 else wv64[6 - 2 * s]
                if pi == 0:
                    return wv64[2 * s]
                return -wv64[2 * s + 1]

            # ---- PE weights: c * I for each (input, parity, tap); F32R-typed
            # so the BIR verifier accepts them as f32r matmul operands.
            it = constp.tile([P, P], F32)
            nc.sync.dma_start(it[:], ident[:, :])
            w16 = constp.tile([P, 16 * P], F32R)

            def wslice(x, pi, s):
                j = (x * 2 + pi) * 4 + s
                return w16[:, j * P : (j + 1) * P]

            for x in range(2):
                for pi in range(2):
                    for s in range(4):
                        nc.vector.tensor_scalar(
                            wslice(x, pi, s), it[:], coeff(x, pi, s), None, MUL
                        )

            taps = [(x, s) for x in range(2) for s in range(4)]
            cvt_op = nc.scalar.copy if CVT_ENG == "scalar" else nc.vector.tensor_copy
            evict_op = nc.scalar.copy if EVICT_ENG == "scalar" else nc.vector.tensor_copy

            def body(_i=None):
              for t in range(NT):
                r0 = t * P
                # F32R-typed so PE can consume windows directly; every
                # non-PE reader bitcasts back to F32 (same bits).
                dext = inp.tile([P, M + 3], F32R, tag="dext")
                nc.sync.dma_start(dext[:, 3 : M + 3], det[r0 : r0 + P, :].bitcast(F32R))
                aext = inp.tile([P, M + 3], F32R, tag="aext")
                nc.sync.dma_start(aext[:, 3 : M + 3], app[r0 : r0 + P, :].bitcast(F32R))
                nc.vector.tensor_copy(dext[:, 0:3], dext[:, M : M + 3])
                nc.vector.tensor_copy(aext[:, 0:3], aext[:, M : M + 3])
                ext = [dext, aext]

                def xf(x, a, b):
                    return ext[x][:, a:b].bitcast(F32)

                for p in range(NPAN):  # one output tile per panel
                    if True:
                        c0 = p * WU
                        oh = outp.tile([P, 2 * WU], F32, tag="out",
                                       name=f"oh_{t}_{p}")
                        if IO_ONLY:
                            for pi in range(2):
                                nc.scalar.copy(
                                    oh[:, pi : 2 * WU : 2],
                                    xf(0, 3 + c0, 3 + c0 + WU),
                                )
                            continue
                        if (t, p) in dve_set:
                            # fp16 aligned copies: bufE = ext[c0:...], bufO = ext[c0+1:...]
                            bE, bO = [], []
                            for x in range(2):
                                be = cvtp.tile([P, WU + 2], F16, tag="cvt")
                                cvt_op(be[:], xf(x, c0, c0 + WU + 2))
                                bo = cvtp.tile([P, WU + 2], F16, tag="cvt")
                                cvt_op(bo[:], xf(x, c0 + 1, c0 + WU + 3))
                                bE.append(be)
                                bO.append(bo)

                            def win(x, s):
                                # tap window = ext[3-s+c0 : 3-s+c0+WU]; bufE holds
                                # ext[c0:...], bufO holds ext[c0+1:...] — offsets
                                # 3-s resp. 2-s are always even (4B-aligned fp16).
                                if s in (1, 3):
                                    return bE[x][:, 3 - s : 3 - s + WU]
                                return bO[x][:, 2 - s : 2 - s + WU]

                            for pi in range(2):
                                oview = oh[:, pi : 2 * WU : 2]
                                chain = [(0, 3), (0, 1), (0, 2), (0, 0),
                                         (1, 3), (1, 1), (1, 2)]
                                acc = accp.tile([P, WU], F16, tag="acc")
                                x0, s0 = chain[0]
                                nc.vector.tensor_scalar(
                                    acc[:], win(x0, s0), coeff(x0, pi, s0), None, MUL
                                )
                                for x, s in chain[1:]:
                                    nc.vector.scalar_tensor_tensor(
                                        acc[:], win(x, s), coeff(x, pi, s),
                                        acc[:], MUL, ADD,
                                    )
                                nc.vector.scalar_tensor_tensor(
                                    oview, win(1, 0), coeff(1, pi, 0),
                                    acc[:], MUL, ADD,
                                )
                        else:  # PE panel: f32r identity-scaled matmuls
                            for pi in range(2):
                                ccs = list(range(c0, c0 + WU, NCHUNK))
                                pss = [
                                    psump.tile([P, NCHUNK], F32, tag="ps",
                                               name=f"ps_{t}_{pi}_{p}_{ci}")
                                    for ci in range(len(ccs))
                                ]
                                for j, (x, s) in enumerate(taps):
                                    w = wslice(x, pi, s)
                                    for ci, cc in enumerate(ccs):
                                        rhs = ext[x][:, 3 - s + cc : 3 - s + cc + NCHUNK]
                                        nc.tensor.matmul(
                                            pss[ci][:], w, rhs,
                                            start=(j == 0), stop=(j == len(taps) - 1),
                                        )
                                for ci, cc in enumerate(ccs):
                                    evb = 2 * (cc - c0) + pi
                                    evict_op(
                                        oh[:, evb : evb + 2 * NCHUNK - 1 : 2], pss[ci][:]
                                    )
                    nc.sync.dma_start(
                        res[r0 : r0 + P, 2 * c0 : 2 * c0 + 2 * WU], oh[:])

            if REPS == 1:
                body()
            else:
                with tc.For_i(0, REPS, 1) as _rv:
                    body(_rv)
    nc.finalize()
    return nc


_CACHE = {}


def _get_nc(wavelet):
    key = wavelet.tobytes()
    if _CACHE.get("key") != key:
        _CACHE["nc"] = build_nc(_dve_panels(), wavelet)
        _CACHE["key"] = key
    return _CACHE["nc"]


def kernel(details, approximation, wavelet):
    details = np.ascontiguousarray(np.asarray(details, dtype=np.float32))
    approximation = np.ascontiguousarray(np.asarray(approximation, dtype=np.float32))
    wavelet = np.ascontiguousarray(np.asarray(wavelet, dtype=np.float32))
    assert details.shape == (N_CORES * ROWS, M) and approximation.shape == details.shape
    assert wavelet.shape == (8,)

    in_maps = [
        {
            "details": details[c * ROWS : (c + 1) * ROWS],
            "approximation": approximation[c * ROWS : (c + 1) * ROWS],
            "wavelet": wavelet,
        }
        for c in range(N_CORES)
    ]
    trace = bool(int(os.environ.get("DWT_TRACE", "0")))
    r = run_bass_kernel_spmd(_get_nc(wavelet), in_maps, list(range(N_CORES)), trace=trace)
    _CACHE["last_results"] = r
    return np.concatenate([r.results[c]["result"] for c in range(N_CORES)], axis=0)


# revision 9
# speedup vs baseline: 1.0055x; 1.0010x over previous
"""Inverse DWT (BackwardTransformLayer) Trainium2 Bass kernel — v2.

Math (polyphase form of the zero-interleaved circular FFT convolution):
  out[r, 2p+pi] = sum_{s=0..3} cD[pi,s]*D[r,(p-s)%M] + cA[pi,s]*A[r,(p-s)%M]
  cD[0,s] = w[7-2s]   cD[1,s] = w[6-2s]   cA[0,s] = w[2s]   cA[1,s] = -w[2s+1]

Sharding: data-parallel over rows; 512 rows per core on 8 NeuronCores.

Measured 96694 ns (REPS-slope method; all-fp32 predecessor: 235940 ns by
the same method), rel err 1.3e-3 vs the 2e-2 gate — essentially at the
~93us/core HBM floor (33.5 MB at ~360 GB/s). The wins, in order:
  - Per-PANEL output tiles [128 x 2048], DMA'd to HBM the moment each
    panel's two parity chains/evictions finish (instead of half-tile
    [128 x 4096] outputs gated on two panels). This single overlap fix was
    worth 30us (126965 -> 96694): finer store granularity keeps the DMA
    queue fed and shrinks the pipeline drain. The optimum is sharp:
    half-tile 16KB stores = 126965, panel 8KB = 96694, 4KB slices =
    126872, splitting INPUT loads the same way = 119725, and deeper
    pools (ine 3 / oute 8) = 126342. Seven measured perturbations all
    fall off a cliff to ~120-142us: this exact configuration is a sharp
    scheduler optimum — change nothing without a hardware A/B.
  - PE panels (10 of 16): f32r (tf32-like) matmuls of identity-scaled
    weights reading the fp32 input tiles directly via bitcast — 1 cyc/row
    instead of fp32's 4, no conversion pass. Tiles are F32R-typed and DMA'd
    via a bitcast source AP because the BIR verifier requires f32r matmul
    operands to be f32r-typed ("rounded"); every other reader bitcasts
    back to F32.
  - Engine rebalance: 6 DVE panels / 10 PE panels (the old 21/11 split left
    DVE as a ~190us critical path). DVE chains are fp16
    scalar_tensor_tensor MACs: on real HW fp16 STT runs in 2x packed mode
    (proved by A/B: identical kernel with fp32 chains and no conversions
    measures 126904 vs 96694 — the CoreSim cost model wrongly says 1x).
    The 4B-alignment dance below is what keeps 2x legal on every window.
    Coefficients are compile-time immediates: the NEFF is specialized on
    the runtime wavelet values by kernel() (works for any wavelet).
  - ScalarE produces two fp16 copies per DVE panel (bufE at ext[c0], bufO
    at ext[c0+1], windows at even offsets) and evicts PE PSUM stride-2.
Tried and measured SLOWER (or neutral), do not redo without new evidence:
  - DVE/PE rebalance: NDVE=5: 140908, NDVE=7: 134879 (6 is the optimum;
    TimelineSim predicted the opposite order — its scheduling deltas do
    NOT transfer to HW).
  - Grouped-PE (taps outermost per tile-parity to amortize LDWEIGHTS):
    128671 — LDW is hidden by the PE engine queue.
  - tensor_scalar(4x-claimed)+tensor_tensor tree chains, input-DMA halving,
    prefix-from-HBM, INP/OUT_BUFS 3/3: CoreSim said 112.8us, HW said
    142.0us — the cost model's fast-mode table for TensorScalarPtr
    overestimates real HW.
  - Parity-granular DVE/PE assignment: 126-129us in sim, never beat
    whole-panel granularity.
"""

import os
import sys

import numpy as np

for _p in ("/opt/trn_rl_repo", "/root/.axon_site/_ro/trn_rl_repo"):
    if os.path.isdir(_p) and _p not in sys.path:
        sys.path.append(_p)

import concourse.bass as bass  # noqa: E402
import concourse.tile as tile  # noqa: E402
from concourse import bacc, mybir  # noqa: E402
from concourse.bass_utils import run_bass_kernel_spmd  # noqa: E402

F32 = mybir.dt.float32
F32R = mybir.dt.float32r
F16 = mybir.dt.float16
COPY = mybir.ActivationFunctionType.Copy
MUL = mybir.AluOpType.mult
ADD = mybir.AluOpType.add

N_CORES = 8
P = 128          # partitions
M = 4096         # input row length
ROWS = 512       # rows per core
NT = ROWS // P   # row tiles per core
WU = 1024        # panel width (input cols)
NPAN = M // WU   # panels per tile
NCHUNK = 512     # psum chunk (one bank of fp32)

NDVE = int(os.environ.get("DWT_NDVE", "6"))     # panels (of NT*NPAN=16) on DVE
REPS = int(os.environ.get("DWT_REPS", "1"))     # benchmark-only in-kernel loop
IO_ONLY = bool(int(os.environ.get("DWT_IO_ONLY", "0")))
EVICT_ENG = os.environ.get("DWT_EVICT", "scalar")  # psum eviction engine
CVT_ENG = os.environ.get("DWT_CVT", "scalar")      # fp16 conversion engine


def _dve_panels():
    # Spread DVE panels across tiles and halves.
    order = [(t, p) for p in (1, 3, 0, 2) for t in range(NT)]
    return set(order[:NDVE])


def build_nc(dve_set, wavelet_vals=None):
    if wavelet_vals is None:
        # DB4 defaults (reference.setup_inputs uses these); kernel() always
        # rebuilds with the actual runtime wavelet on first call.
        wavelet_vals = np.array([-0.010597401784997278, 0.032883011666982945,
                                 0.030841381835986965, -0.18703481171888114,
                                 -0.02798376941698385, 0.6308807679295904,
                                 0.7148465705525415, 0.23037781330885523],
                                dtype=np.float64)
    wv64 = [float(v) for v in np.asarray(wavelet_vals, dtype=np.float64)]
    nc = bacc.Bacc()
    det = nc.declare_dram_parameter("details", [ROWS, M], F32, isOutput=False)
    app = nc.declare_dram_parameter("approximation", [ROWS, M], F32, isOutput=False)
    wav = nc.declare_dram_parameter("wavelet", [8], F32, isOutput=False)
    res = nc.declare_dram_parameter("result", [ROWS, 2 * M], F32, isOutput=True)
    ident = nc.inline_tensor(np.eye(P, dtype=np.float32), "ident")

    with tile.TileContext(nc) as tc:
        with (
            tc.tile_pool(name="const", bufs=1) as constp,
            tc.tile_pool(name="ine", bufs=2) as inp,
            tc.tile_pool(name="oute", bufs=6) as outp,
            tc.tile_pool(name="cvt", bufs=8) as cvtp,
            tc.tile_pool(name="acc", bufs=4) as accp,
            tc.tile_pool(name="psum", bufs=8, space="PSUM") as psump,
        ):
            # ---- coefficients as compile-time immediates: the scalar op
            # then lowers to TensorScalar (not TensorScalarPtr), whose 2x/4x
            # DVE uops exist; the Ptr variant runs at 1x. The NEFF is
            # specialized on the runtime wavelet values by kernel().
            # Token read keeps the "wavelet" ExternalInput alive in the NEFF.
            wv = constp.tile([1, 8], F32)
            nc.sync.dma_start(wv[:], wav[None, :])

            def coeff(x, pi, s):
                # x: 0 = details, 1 = approximation; pi: 0 = even, 1 = odd
                if x == 0:
                    return wv64[7 - 2 * s] if pi == 0 else wv64[6 - 2 * s]
                if pi == 0:
                    return wv64[2 * s]
                return -wv64[2 * s + 1]

            # ---- PE weights: c * I for each (input, parity, tap); F32R-typed
            # so the BIR verifier accepts them as f32r matmul operands.
            it = constp.tile([P, P], F32)
            nc.sync.dma_start(it[:], ident[:, :])
            w16 = constp.tile([P, 16 * P], F32R)

            def wslice(x, pi, s):
                j = (x * 2 + pi) * 4 + s
                return w16[:, j * P : (j + 1) * P]

            for x in range(2):
                for pi in range(2):
                    for s in range(4):
                        nc.vector.tensor_scalar(
                            wslice(x, pi, s), it[:], coeff(x, pi, s), None, MUL
                        )

            taps = [(x, s) for x in range(2) for s in range(4)]
            cvt_op = nc.scalar.copy if CVT_ENG == "scalar" else nc.vector.tensor_copy
            evict_op = nc.scalar.copy if EVICT_ENG == "scalar" else nc.vector.tensor_copy

            def body(_i=None):
              for t in range(NT):
                r0 = t * P
                # F32R-typed so PE can consume windows directly; every
                # non-PE reader bitcasts back to F32 (same bits).
                dext = inp.tile([P, M + 3], F32R, tag="dext")
                nc.sync.dma_start(dext[:, 3 : M + 3], det[r0 : r0 + P, :].bitcast(F32R))
                aext = inp.tile([P, M + 3], F32R, tag="aext")
                nc.sync.dma_start(aext[:, 3 : M + 3], app[r0 : r0 + P, :].bitcast(F32R))
                nc.vector.tensor_copy(dext[:, 0:3], dext[:, M : M + 3])
                nc.vector.tensor_copy(aext[:, 0:3], aext[:, M : M + 3])
                ext = [dext, aext]

                def xf(x, a, b):
                    return ext[x][:, a:b].bitcast(F32)

                for p in range(NPAN):  # one output tile per panel
                    if True:
                        c0 = p * WU
                        oh = outp.tile([P, 2 * WU], F32, tag="out",
                                       name=f"oh_{t}_{p}")
                        if IO_ONLY:
                            for pi in range(2):
                                nc.scalar.copy(
                                    oh[:, pi : 2 * WU : 2],
                                    xf(0, 3 + c0, 3 + c0 + WU),
                                )
                            continue
                        if (t, p) in dve_set:
                            # fp16 aligned copies: bufE = ext[c0:...], bufO = ext[c0+1:...]
                            bE, bO = [], []
                            for x in range(2):
                                be = cvtp.tile([P, WU + 2], F16, tag="cvt")
                                cvt_op(be[:], xf(x, c0, c0 + WU + 2))
                                bo = cvtp.tile([P, WU + 2], F16, tag="cvt")
                                cvt_op(bo[:], xf(x, c0 + 1, c0 + WU + 3))
                                bE.append(be)
                                bO.append(bo)

                            def win(x, s):
                                # tap window = ext[3-s+c0 : 3-s+c0+WU]; bufE holds
                                # ext[c0:...], bufO holds ext[c0+1:...] — offsets
                                # 3-s resp. 2-s are always even (4B-aligned fp16).
                                if s in (1, 3):
                                    return bE[x][:, 3 - s : 3 - s + WU]
                                return bO[x][:, 2 - s : 2 - s + WU]

                            for pi in range(2):
                                oview = oh[:, pi : 2 * WU : 2]
                                chain = [(0, 3), (0, 1), (0, 2), (0, 0),
                                         (1, 3), (1, 1), (1, 2)]
                                acc = accp.tile([P, WU], F16, tag="acc")
                                x0, s0 = chain[0]
                                nc.vector.tensor_scalar(
                                    acc[:], win(x0, s0), coeff(x0, pi, s0), None, MUL
                                )
                                for x, s in chain[1:]:
                                    nc.vector.scalar_tensor_tensor(
                                        acc[:], win(x, s), coeff(x, pi, s),
                                        acc[:], MUL, ADD,
                                    )
                                nc.vector.scalar_tensor_tensor(
                                    oview, win(1, 0), coeff(1, pi, 0),
                                    acc[:], MUL, ADD,
                                )
                        else:  # PE panel: f32r identity-scaled matmuls
                            for pi in range(2):
                                ccs = list(range(c0, c0 + WU, NCHUNK))
                                pss = [
                                    psump.tile([P, NCHUNK], F32, tag="ps",
                                               name=f"ps_{t}_{pi}_{p}_{ci}")
                                    for ci in range(len(ccs))
                                ]
                                for j, (x, s) in enumerate(taps):
                                    w = wslice(x, pi, s)
                                    for ci, cc in enumerate(ccs):
                                        rhs = ext[x][:, 3 - s + cc : 3 - s + cc + NCHUNK]
                                        nc.tensor.matmul(
                                            pss[ci][:], w, rhs,
                                            start=(j == 0), stop=(j == len(taps) - 1),
                                        )
                                for ci, cc in enumerate(ccs):
                                    evb = 2 * (cc - c0) + pi
                                    evict_op(
                                        oh[:, evb : evb + 2 * NCHUNK - 1 : 2], pss[ci][:]
                                    )
                    nc.sync.dma_start(
                        res[r0 : r0 + P, 2 * c0 : 2 * c0 + 2 * WU], oh[:])

            if REPS == 1:
                body()
            else:
                with tc.For_i(0, REPS, 1) as _rv:
                    body(_rv)
    nc.finalize()
    return nc


_CACHE = {}


def _get_nc(wavelet):
    key = wavelet.tobytes()
    if _CACHE.get("key") != key:
        _CACHE["nc"] = build_nc(_dve_panels(), wavelet)
        _CACHE["key"] = key
    return _CACHE["nc"]


def kernel(details, approximation, wavelet):
    details = np.ascontiguousarray(np.asarray(details, dtype=np.float32))
    approximation = np.ascontiguousarray(np.asarray(approximation, dtype=np.float32))
    wavelet = np.ascontiguousarray(np.asarray(wavelet, dtype=np.float32))
    assert details.shape == (N_CORES * ROWS, M) and approximation.shape == details.shape
    assert wavelet.shape == (8,)

    in_maps = [
        {
            "details": details[c * ROWS : (c + 1) * ROWS],
            "approximation": approximation[c * ROWS : (c + 1) * ROWS],
            "wavelet": wavelet,
        }
        for c in range(N_CORES)
    ]
    trace = bool(int(os.environ.get("DWT_TRACE", "0")))
    r = run_bass_kernel_spmd(_get_nc(wavelet), in_maps, list(range(N_CORES)), trace=trace)
    _CACHE["last_results"] = r
    return np.concatenate([r.results[c]["result"] for c in range(N_CORES)], axis=0)
